# revision 1
# baseline (speedup 1.0000x reference)
"""Trainium2 Bass kernel for neighbor-sum aggregation (GNN message passing).

reference:  out[b, :] = sum_k embed_matrix[neigh_idx[b, k], :]   (B=50000, K=16,
            U=100000, D=512)

Strategy: data-parallel over B across 8 NeuronCores (embed_matrix replicated).
Each core processes 6272 rows (B padded 50000 -> 50176 with dummy index-0 rows)
as 49 tiles of 128 rows. Per tile, ONE SWDGE indirect DMA gathers all 128x16
neighbor rows (2048 descriptors, 4 MB) into an SBUF tile laid out
[128 partitions, 16*512 f32]; a 4-level in-place tree of DVE adds reduces the
16 chunks to the 512-wide output row, which is stored with a HWDGE DMA.
"""

import numpy as np

import concourse.bacc as bacc
import concourse.bass as bass
import concourse.mybir as mybir
import concourse.tile as tile
from concourse.bass_utils import run_bass_kernel_spmd

N_CORES = 8
B, K = 50000, 16
U, D = 100000, 512
P = 128
TILES = 49                      # output tiles per core
B_SHARD = TILES * P             # 6272 padded rows per core
B_PAD = N_CORES * B_SHARD       # 50176

GATH_BUFS = 3
ACC_BUFS = 3

_NC_CACHE = {}


def build_nc(reps=1):
    """reps>1 wraps the whole tile loop in a hardware For_i for benchmarking."""
    nc = bacc.Bacc("TRN2", target_bir_lowering=False, debug=False)
    idx = nc.dram_tensor("idx", [B_SHARD, K], mybir.dt.int32, kind="ExternalInput")
    embed = nc.dram_tensor("embed", [U, D], mybir.dt.float32, kind="ExternalInput")
    out = nc.dram_tensor("out", [B_SHARD, D], mybir.dt.float32, kind="ExternalOutput")

    with tile.TileContext(nc) as tc:
        with (
            tc.tile_pool(name="idxp", bufs=1) as idx_pool,
            tc.tile_pool(name="gath", bufs=GATH_BUFS) as gpool,
            tc.tile_pool(name="accp", bufs=ACC_BUFS) as apool,
        ):
            # All indices up-front in one DMA: idx_all[p, t*K+k] = idx[t*128+p, k]
            idx_all = idx_pool.tile([P, TILES * K], mybir.dt.int32)
            nc.sync.dma_start(
                out=idx_all[:].rearrange("p (t k) -> p t k", k=K),
                in_=idx.ap().rearrange("(t p) k -> p t k", p=P),
            )

            def body():
                for t in range(TILES):
                    gath = gpool.tile([P, K * D], mybir.dt.float32, tag="g")
                    # HW indirect DMA: exactly one index per partition per op,
                    # each gathering one contiguous D-row of embed.
                    for k in range(K):
                        nc.gpsimd.indirect_dma_start(
                            out=gath[:, k * D : (k + 1) * D],
                            out_offset=None,
                            in_=embed.ap(),
                            in_offset=bass.IndirectOffsetOnAxis(
                                ap=idx_all[:, t * K + k : t * K + k + 1], axis=0
                            ),
                        )
                    # Single-port DVE reduce over the strided [p][d][k] view —
                    # avoids 2-port tensor_tensor ops that contend with the
                    # GpSimd SWDGE descriptor path for the shared SBUF port.
                    acc = apool.tile([P, D], mybir.dt.float32, tag="a")
                    nc.vector.tensor_reduce(
                        out=acc[:],
                        in_=gath[:].rearrange("p (k d) -> p d k", d=D),
                        axis=mybir.AxisListType.X,
                        op=mybir.AluOpType.add,
                    )
                    nc.sync.dma_start(
                        out=out.ap()[t * P : (t + 1) * P, :], in_=acc[:]
                    )

            if reps == 1:
                body()
            else:
                with tc.For_i(0, reps, 1):
                    body()
    nc.compile()
    return nc


def _get_nc():
    if "nc" not in _NC_CACHE:
        _NC_CACHE["nc"] = build_nc()
    return _NC_CACHE["nc"]


def _run(nc, in_maps, **kwargs):
    return run_bass_kernel_spmd(nc, in_maps, list(range(N_CORES)), **kwargs)


def make_in_maps(neigh_idx, embed_matrix):
    idx = np.asarray(neigh_idx).astype(np.int32)
    embed = np.ascontiguousarray(np.asarray(embed_matrix), dtype=np.float32)
    idx_pad = np.zeros((B_PAD, K), np.int32)
    idx_pad[:B] = idx
    shards = idx_pad.reshape(N_CORES, B_SHARD, K)
    return [
        {"idx": np.ascontiguousarray(shards[c]), "embed": embed}
        for c in range(N_CORES)
    ]


def kernel(neigh_idx, embed_matrix):
    nc = _get_nc()
    in_maps = make_in_maps(neigh_idx, embed_matrix)
    res = _run(nc, in_maps).results
    out = np.concatenate([res[c]["out"] for c in range(N_CORES)], axis=0)[:B]
    return np.ascontiguousarray(out, dtype=np.float32)



# revision 3
# speedup vs baseline: 2.3193x; 2.3193x over previous
"""Trainium2 Bass kernel for neighbor-sum aggregation (GNN message passing).

reference:  out[b, :] = sum_k embed_matrix[neigh_idx[b, k], :]   (B=50000, K=16,
            U=100000, D=512)

Strategy: data-parallel over B across 8 NeuronCores (embed_matrix replicated).
Each core processes 6272 rows (B padded 50000 -> 50176 with dummy index-0 rows)
as 49 tiles of 128 rows. Per tile, ONE SWDGE indirect DMA gathers all 128x16
neighbor rows (2048 descriptors, 4 MB) into an SBUF tile laid out
[128 partitions, 16*512 f32]; a 4-level in-place tree of DVE adds reduces the
16 chunks to the 512-wide output row, which is stored with a HWDGE DMA.
"""

import numpy as np

import concourse.bacc as bacc
import concourse.bass as bass
import concourse.mybir as mybir
import concourse.tile as tile
from concourse.bass_utils import run_bass_kernel_spmd

N_CORES = 8
B, K = 50000, 16
U, D = 100000, 512
P = 128
TILES = 49                      # output tiles per core
B_SHARD = TILES * P             # 6272 padded rows per core
B_PAD = N_CORES * B_SHARD       # 50176

GATH_BUFS = 3
ACC_BUFS = 3

_NC_CACHE = {}


def build_nc(reps=1):
    """reps>1 wraps the whole tile loop in a hardware For_i for benchmarking."""
    nc = bacc.Bacc("TRN2", target_bir_lowering=False, debug=False)
    idx = nc.dram_tensor("idx", [B_SHARD, K], mybir.dt.int32, kind="ExternalInput")
    embed = nc.dram_tensor("embed", [U, D], mybir.dt.float32, kind="ExternalInput")
    out = nc.dram_tensor("out", [B_SHARD, D], mybir.dt.float32, kind="ExternalOutput")

    with tile.TileContext(nc) as tc:
        with (
            tc.tile_pool(name="idxp", bufs=1) as idx_pool,
            tc.tile_pool(name="gath", bufs=GATH_BUFS) as gpool,
            tc.tile_pool(name="accp", bufs=ACC_BUFS) as apool,
        ):
            # All indices up-front in one DMA: idx_all[p, t*K+k] = idx[t*128+p, k]
            idx_all = idx_pool.tile([P, TILES * K], mybir.dt.int32)
            nc.sync.dma_start(
                out=idx_all[:].rearrange("p (t k) -> p t k", k=K),
                in_=idx.ap().rearrange("(t p) k -> p t k", p=P),
            )

            def body():
                for t in range(TILES):
                    gath = gpool.tile([P, K * D], mybir.dt.float32, tag="g")
                    # HW indirect DMA: exactly one index per partition per op,
                    # each gathering one contiguous D-row of embed.
                    for k in range(K):
                        nc.gpsimd.indirect_dma_start(
                            out=gath[:, k * D : (k + 1) * D],
                            out_offset=None,
                            in_=embed.ap(),
                            in_offset=bass.IndirectOffsetOnAxis(
                                ap=idx_all[:, t * K + k : t * K + k + 1], axis=0
                            ),
                        )
                    # Single-port DVE reduce over the strided [p][d][k] view —
                    # avoids 2-port tensor_tensor ops that contend with the
                    # GpSimd SWDGE descriptor path for the shared SBUF port.
                    acc = apool.tile([P, D], mybir.dt.float32, tag="a")
                    nc.vector.tensor_reduce(
                        out=acc[:],
                        in_=gath[:].rearrange("p (k d) -> p d k", d=D),
                        axis=mybir.AxisListType.X,
                        op=mybir.AluOpType.add,
                    )
                    nc.sync.dma_start(
                        out=out.ap()[t * P : (t + 1) * P, :], in_=acc[:]
                    )

            if reps == 1:
                body()
            else:
                with tc.For_i(0, reps, 1):
                    body()
    nc.compile()
    return nc


def _get_nc():
    if "nc" not in _NC_CACHE:
        _NC_CACHE["nc"] = build_nc()
    return _NC_CACHE["nc"]


def _run(nc, in_maps, **kwargs):
    return run_bass_kernel_spmd(nc, in_maps, list(range(N_CORES)), **kwargs)


def make_in_maps(neigh_idx, embed_matrix):
    idx = np.asarray(neigh_idx).astype(np.int32)
    embed = np.ascontiguousarray(np.asarray(embed_matrix), dtype=np.float32)
    idx_pad = np.zeros((B_PAD, K), np.int32)
    idx_pad[:B] = idx
    shards = idx_pad.reshape(N_CORES, B_SHARD, K)
    return [
        {"idx": np.ascontiguousarray(shards[c]), "embed": embed}
        for c in range(N_CORES)
    ]


def kernel(neigh_idx, embed_matrix):
    nc = _get_nc()
    in_maps = make_in_maps(neigh_idx, embed_matrix)
    res = _run(nc, in_maps).results
    out = np.concatenate([res[c]["out"] for c in range(N_CORES)], axis=0)[:B]
    return np.ascontiguousarray(out, dtype=np.float32)



# revision 4
# speedup vs baseline: 2.3652x; 1.0198x over previous
"""Trainium2 Bass kernel for neighbor-sum aggregation (GNN message passing), v4.

reference:  out[b, :] = sum_k embed_matrix[neigh_idx[b, k], :]   (B=50000, K=16,
            U=100000, D=512)

v4 = v3b + host-side tile rebalancing. Rows are bin-packed into 50 tiles of
128 rows (greedy 4-dim balance) so that every (tile, class) gather op carries
at most 512 tokens -> exactly 4 slots per class, 16 slots per tile (vs 20).
This removes the 5th "variance" slot per class: 20% fewer matmuls and 20%
fewer PE gbuf reads (the dominant SBUF traffic alongside the gather writes).
The row permutation is undone on the host after the run.

Pipeline per tile: 4 dma_gather ops (one per index class u%4, int16 local
indices u//4 into a 4-row-strided view of bf16 embed, exact runtime counts
via reg_load, -1 tails skipped) fill a 16-slot bf16 buffer; 16 accumulating
matmuls with host-built one-hot fp8 selectors route gathered tokens into a
[128, 512] fp32 PSUM bank; DVE copies to bf16; HWDGE stores; host upcasts.
4 SWDGE queues (one per class) + 64 KB descriptor scratch keep descriptor
generation off the critical path.
"""

import ml_dtypes
import numpy as np

import concourse.bacc as bacc
import concourse.bass as bass
import concourse.mybir as mybir
import concourse.tile as tile
from concourse.bass_utils import run_bass_kernel_spmd

N_CORES = 8
B, K = 50000, 16
U, D = 100000, 512
P = 128
TILES = 50                      # output tiles per core (rebalanced)
B_SHARD = TILES * P             # 6400 device rows per core
B_REAL = 6250                   # real rows per core

NCLS = 4                        # index classes (int16 range: u//4 < 25000)
SLOTS_PER_CLS = 4               # 512-token capacity per (tile, class) op
NI = SLOTS_PER_CLS * P          # num_idxs per gather op
COLS = NI // 16                 # idx16 columns per op
N_OPS = TILES * NCLS
SLOTS = NCLS * SLOTS_PER_CLS    # gather slots per tile

GATH_BUFS = 3
SEL_BUFS = 3
OUT_BUFS = 3
PSUM_BUFS = 4

FP8 = mybir.dt.float8e4
_NC_CACHE = {}


def build_nc(reps=1, sim_zero_gbuf=False):
    """reps>1 wraps the whole tile loop in a hardware For_i for benchmarking."""
    nc = bacc.Bacc(
        "TRN2", target_bir_lowering=False, debug=False, num_swdge_queues=4,
        dynamic_dma_scratch_size=65536,
    )
    idx16 = nc.dram_tensor(
        "idx16", [P, N_OPS * COLS], mybir.dt.int16, kind="ExternalInput"
    )
    counts = nc.dram_tensor("counts", [1, N_OPS], mybir.dt.int32, kind="ExternalInput")
    sel = nc.dram_tensor("sel", [TILES, P, SLOTS * P], FP8, kind="ExternalInput")
    embed = nc.dram_tensor("embed", [U, D], mybir.dt.bfloat16, kind="ExternalInput")
    out = nc.dram_tensor("out", [B_SHARD, D], mybir.dt.bfloat16, kind="ExternalOutput")

    with tile.TileContext(nc) as tc:
        with (
            tc.tile_pool(name="cst", bufs=1) as cpool,
            tc.tile_pool(name="gath", bufs=GATH_BUFS) as gpool,
            tc.tile_pool(name="selp", bufs=SEL_BUFS) as spool,
            tc.tile_pool(name="outp", bufs=OUT_BUFS) as opool,
            tc.tile_pool(name="ps", bufs=PSUM_BUFS, space=bass.MemorySpace.PSUM) as ppool,
        ):
            idx_all = cpool.tile([P, N_OPS * COLS], mybir.dt.int16)
            nc.sync.dma_start(out=idx_all[:], in_=idx16.ap())
            cnt_sb = cpool.tile([1, N_OPS], mybir.dt.int32)
            nc.sync.dma_start(out=cnt_sb[:], in_=counts.ap())
            cnt_regs = [
                nc.alloc_register(mybir.EngineType.Pool, f"cnt{q}")
                for q in range(NCLS)
            ]
            embed_cls = embed.ap().rearrange("(u q) d -> u q d", q=NCLS)

            # Zero the rotating gather bufs once: -1 index tails skip their
            # slots, and a matmul reading uninitialized SBUF (NaN) would
            # poison the PSUM even against zero sel columns (NaN * 0 = NaN).
            for _ in range(GATH_BUFS):
                gz = gpool.tile([P, SLOTS * D], mybir.dt.bfloat16, tag="g")
                nc.vector.memset(gz[:], 0.0)

            def body():
                for t in range(TILES):
                    sel_sb = spool.tile([P, SLOTS * P], FP8, tag="s")
                    nc.sync.dma_start(out=sel_sb[:], in_=sel.ap()[t])
                    gbuf = gpool.tile([P, SLOTS * D], mybir.dt.bfloat16, tag="g")
                    if sim_zero_gbuf:
                        # CoreSim virtualizes pool tiles per allocation and
                        # cannot see the physical prelude zeroing; only used
                        # for simulation-side validation.
                        nc.vector.memset(gbuf[:], 0.0)
                    gbuf3 = gbuf[:].rearrange("p (s d) -> p s d", d=D)
                    for q in range(NCLS):
                        op = t * NCLS + q
                        nc.gpsimd.reg_load(cnt_regs[q], cnt_sb[0:1, op : op + 1])
                        nc.gpsimd.dma_gather(
                            out_ap=gbuf3[
                                :, q * SLOTS_PER_CLS : (q + 1) * SLOTS_PER_CLS, :
                            ],
                            in_ap=embed_cls[:, q, :],
                            idxs_ap=idx_all[:, op * COLS : (op + 1) * COLS],
                            num_idxs=NI,
                            num_idxs_reg=cnt_regs[q],
                            elem_size=D,
                            elem_step=NCLS * D,
                            queue_num=q,
                        )
                    ps = ppool.tile([P, D], mybir.dt.float32, tag="ps")
                    for sg in range(SLOTS):
                        nc.tensor.matmul(
                            ps[:],
                            sel_sb[:, sg * P : (sg + 1) * P],
                            gbuf[:, sg * D : (sg + 1) * D],
                            start=(sg == 0),
                            stop=(sg == SLOTS - 1),
                        )
                    ob = opool.tile([P, D], mybir.dt.bfloat16, tag="o")
                    nc.vector.tensor_copy(ob[:], ps[:])
                    nc.sync.dma_start(out=out.ap()[t * P : (t + 1) * P, :], in_=ob[:])

            if reps == 1:
                body()
            else:
                with tc.For_i(0, reps, 1):
                    body()
    nc.compile()
    return nc


def _get_nc():
    if "nc" not in _NC_CACHE:
        _NC_CACHE["nc"] = build_nc()
    return _NC_CACHE["nc"]


def _run(nc, in_maps, **kwargs):
    return run_bass_kernel_spmd(nc, in_maps, list(range(N_CORES)), **kwargs)


def _balance(shard):
    """Greedy 4-dim balanced bin packing: assign each of B_REAL rows to one of
    TILES bins of <=128 rows such that every (bin, class) token count <= NI.
    Returns pos[r] = device row of shard row r."""
    cnt = np.stack([(shard & 3 == q).sum(1) for q in range(NCLS)], 1)  # [B_REAL, 4]
    order = np.argsort(-cnt.max(1), kind="stable")
    bin_cnt = np.zeros((TILES, NCLS), np.int32)
    bin_rows = np.zeros(TILES, np.int32)
    assign = np.empty(B_REAL, np.int32)
    for r in order:
        c4 = cnt[r]
        cand = bin_cnt + c4
        feas = (bin_rows < P) & (cand <= NI).all(1)
        if not feas.any():
            raise ValueError("tile balancing infeasible")
        load = cand.max(1).astype(np.float64) + bin_rows / 1000.0
        load[~feas] = 1e9
        b = int(load.argmin())
        assign[r] = b
        bin_cnt[b] += c4
        bin_rows[b] += 1
    # device position: rows of bin b occupy b*128.. in assignment order
    pos = np.empty(B_REAL, np.int64)
    next_slot = (np.arange(TILES) * P).astype(np.int64)
    for r in range(B_REAL):
        b = assign[r]
        pos[r] = next_slot[b]
        next_slot[b] += 1
    return pos


def _prep_core(shard):
    """shard: [B_REAL, K] int64 indices -> (idx16, sel, counts, pos)."""
    fp8_np = mybir.dt.np(FP8)
    pos = _balance(shard)
    idx16 = np.zeros((P, N_OPS * COLS), np.int16)
    sel = np.zeros((TILES, P, SLOTS * P), np.float32)
    cnts = np.zeros(N_OPS, np.int32)

    dev_tile = pos // P          # tile of each real row
    dev_loc = pos % P            # row-within-tile
    cls = (shard & 3).astype(np.int8)          # [B_REAL, K]
    loc16 = (shard >> 2).astype(np.int16)
    for t in range(TILES):
        rmask = dev_tile == t
        rids = np.nonzero(rmask)[0]
        for q in range(NCLS):
            rr, kk = np.nonzero(cls[rids] == q)
            n = rr.size
            if n > NI:
                raise ValueError(f"class overflow: tile {t} class {q}: {n}")
            loc = loc16[rids[rr], kk]
            rows_local = dev_loc[rids[rr]]
            order = np.argsort(loc, kind="stable")
            loc, rows_local = loc[order], rows_local[order]
            g = np.full(NI, -1, np.int16)
            g[:n] = loc
            op = t * NCLS + q
            cnts[op] = n
            idx16[:, op * COLS : (op + 1) * COLS] = np.tile(
                g.reshape(COLS, 16).T, (8, 1)
            )
            i = np.arange(n)
            sg = q * SLOTS_PER_CLS + i // P
            sel[t, i % P, sg * P + rows_local] = 1.0
    return idx16, sel.astype(fp8_np), cnts.reshape(1, N_OPS), pos


def _prep_all(neigh_idx):
    idx = np.asarray(neigh_idx).astype(np.int64)
    pad = np.broadcast_to(
        np.arange(K, dtype=np.int64) % NCLS, (N_CORES * B_REAL, K)
    ).copy()
    pad[:B] = idx
    shards = pad.reshape(N_CORES, B_REAL, K)
    in_maps, poss = [], []
    for c in range(N_CORES):
        idx16, sel, cnts, pos = _prep_core(shards[c])
        in_maps.append({"idx16": idx16, "sel": sel, "counts": cnts})
        poss.append(pos)
    return in_maps, poss


def make_in_maps(neigh_idx, embed_matrix):
    embed = np.ascontiguousarray(
        np.asarray(embed_matrix).astype(ml_dtypes.bfloat16)
    )
    in_maps, _ = _prep_all(neigh_idx)
    for m in in_maps:
        m["embed"] = embed
    return in_maps


def kernel(neigh_idx, embed_matrix):
    nc = _get_nc()
    embed = np.ascontiguousarray(
        np.asarray(embed_matrix).astype(ml_dtypes.bfloat16)
    )
    in_maps, poss = _prep_all(neigh_idx)
    for m in in_maps:
        m["embed"] = embed
    res = _run(nc, in_maps).results
    out = np.empty((B, D), np.float32)
    for c in range(N_CORES):
        dev = np.asarray(res[c]["out"]).astype(np.float32)   # [B_SHARD, D]
        lo, hi = c * B_REAL, min((c + 1) * B_REAL, B)
        out[lo:hi] = dev[poss[c][: hi - lo]]
    return out


# revision 6
# speedup vs baseline: 2.3806x; 1.0065x over previous
"""Trainium2 Bass kernel for neighbor-sum aggregation (GNN message passing), v4.

reference:  out[b, :] = sum_k embed_matrix[neigh_idx[b, k], :]   (B=50000, K=16,
            U=100000, D=512)

v4 = v3b + host-side tile rebalancing. Rows are bin-packed into 50 tiles of
128 rows (greedy 4-dim balance) so that every (tile, class) gather op carries
at most 512 tokens -> exactly 4 slots per class, 16 slots per tile (vs 20).
This removes the 5th "variance" slot per class: 20% fewer matmuls and 20%
fewer PE gbuf reads (the dominant SBUF traffic alongside the gather writes).
The row permutation is undone on the host after the run.

Pipeline per tile: 4 dma_gather ops (one per index class u%4, int16 local
indices u//4 into a 4-row-strided view of bf16 embed, exact runtime counts
via reg_load, -1 tails skipped) fill a 16-slot bf16 buffer; 16 accumulating
matmuls with host-built one-hot fp8 selectors route gathered tokens into a
[128, 512] fp32 PSUM bank; DVE copies to bf16; HWDGE stores; host upcasts.
4 SWDGE queues (one per class) + 64 KB descriptor scratch keep descriptor
generation off the critical path.
"""

import ml_dtypes
import numpy as np

import concourse.bacc as bacc
import concourse.bass as bass
import concourse.mybir as mybir
import concourse.tile as tile
from concourse.bass_utils import run_bass_kernel_spmd

N_CORES = 8
B, K = 50000, 16
U, D = 100000, 512
P = 128
TILES = 50                      # output tiles per core (rebalanced)
B_SHARD = TILES * P             # 6400 device rows per core
B_REAL = 6250                   # real rows per core

NCLS = 4                        # index classes (int16 range: u//4 < 25000)
SLOTS_PER_CLS = 4               # 512-token capacity per (tile, class) op
NI = SLOTS_PER_CLS * P          # num_idxs per gather op
COLS = NI // 16                 # idx16 columns per op
N_OPS = TILES * NCLS
SLOTS = NCLS * SLOTS_PER_CLS    # gather slots per tile

GATH_BUFS = 4
SEL_BUFS = 4
OUT_BUFS = 3
PSUM_BUFS = 6

FP8 = mybir.dt.float8e4
_NC_CACHE = {}


def build_nc(reps=1, sim_zero_gbuf=False, gath_bufs=GATH_BUFS, sel_bufs=SEL_BUFS,
             psum_bufs=PSUM_BUFS, scratch=65536):
    """reps>1 wraps the whole tile loop in a hardware For_i for benchmarking."""
    nc = bacc.Bacc(
        "TRN2", target_bir_lowering=False, debug=False, num_swdge_queues=4,
        dynamic_dma_scratch_size=scratch,
    )
    idx16 = nc.dram_tensor(
        "idx16", [P, N_OPS * COLS], mybir.dt.int16, kind="ExternalInput"
    )
    counts = nc.dram_tensor("counts", [1, N_OPS], mybir.dt.int32, kind="ExternalInput")
    sel = nc.dram_tensor("sel", [TILES, P, SLOTS * P], FP8, kind="ExternalInput")
    embed = nc.dram_tensor("embed", [U, D], mybir.dt.bfloat16, kind="ExternalInput")
    out = nc.dram_tensor("out", [B_SHARD, D], mybir.dt.bfloat16, kind="ExternalOutput")

    with tile.TileContext(nc) as tc:
        with (
            tc.tile_pool(name="cst", bufs=1) as cpool,
            tc.tile_pool(name="gath", bufs=gath_bufs) as gpool,
            tc.tile_pool(name="selp", bufs=sel_bufs) as spool,
            tc.tile_pool(name="outp", bufs=OUT_BUFS) as opool,
            tc.tile_pool(name="ps", bufs=psum_bufs, space=bass.MemorySpace.PSUM) as ppool,
        ):
            idx_all = cpool.tile([P, N_OPS * COLS], mybir.dt.int16)
            nc.sync.dma_start(out=idx_all[:], in_=idx16.ap())
            cnt_sb = cpool.tile([1, N_OPS], mybir.dt.int32)
            nc.sync.dma_start(out=cnt_sb[:], in_=counts.ap())
            cnt_regs = [
                nc.alloc_register(mybir.EngineType.Pool, f"cnt{q}")
                for q in range(NCLS)
            ]
            embed_cls = embed.ap().rearrange("(u q) d -> u q d", q=NCLS)

            # Zero the rotating gather bufs once: -1 index tails skip their
            # slots, and a matmul reading uninitialized SBUF (NaN) would
            # poison the PSUM even against zero sel columns (NaN * 0 = NaN).
            for _ in range(gath_bufs):
                gz = gpool.tile([P, SLOTS * D], mybir.dt.bfloat16, tag="g")
                nc.vector.memset(gz[:], 0.0)

            def body():
                for t in range(TILES):
                    sel_sb = spool.tile([P, SLOTS * P], FP8, tag="s")
                    nc.sync.dma_start(out=sel_sb[:], in_=sel.ap()[t])
                    gbuf = gpool.tile([P, SLOTS * D], mybir.dt.bfloat16, tag="g")
                    if sim_zero_gbuf:
                        # CoreSim virtualizes pool tiles per allocation and
                        # cannot see the physical prelude zeroing; only used
                        # for simulation-side validation.
                        nc.vector.memset(gbuf[:], 0.0)
                    gbuf3 = gbuf[:].rearrange("p (s d) -> p s d", d=D)
                    for q in range(NCLS):
                        op = t * NCLS + q
                        nc.gpsimd.reg_load(cnt_regs[q], cnt_sb[0:1, op : op + 1])
                        nc.gpsimd.dma_gather(
                            out_ap=gbuf3[
                                :, q * SLOTS_PER_CLS : (q + 1) * SLOTS_PER_CLS, :
                            ],
                            in_ap=embed_cls[:, q, :],
                            idxs_ap=idx_all[:, op * COLS : (op + 1) * COLS],
                            num_idxs=NI,
                            num_idxs_reg=cnt_regs[q],
                            elem_size=D,
                            elem_step=NCLS * D,
                            queue_num=q,
                        )
                    ps = ppool.tile([P, D], mybir.dt.float32, tag="ps")
                    for sg in range(SLOTS):
                        nc.tensor.matmul(
                            ps[:],
                            sel_sb[:, sg * P : (sg + 1) * P],
                            gbuf[:, sg * D : (sg + 1) * D],
                            start=(sg == 0),
                            stop=(sg == SLOTS - 1),
                        )
                    ob = opool.tile([P, D], mybir.dt.bfloat16, tag="o")
                    nc.vector.tensor_copy(ob[:], ps[:])
                    nc.sync.dma_start(out=out.ap()[t * P : (t + 1) * P, :], in_=ob[:])

            if reps == 1:
                body()
            else:
                with tc.For_i(0, reps, 1):
                    body()
    nc.compile()
    return nc


def _get_nc():
    if "nc" not in _NC_CACHE:
        _NC_CACHE["nc"] = build_nc()
    return _NC_CACHE["nc"]


def _run(nc, in_maps, **kwargs):
    return run_bass_kernel_spmd(nc, in_maps, list(range(N_CORES)), **kwargs)


def _balance(shard):
    """Greedy 4-dim balanced bin packing: assign each of B_REAL rows to one of
    TILES bins of <=128 rows such that every (bin, class) token count <= NI.
    Returns pos[r] = device row of shard row r."""
    cnt = np.stack([(shard & 3 == q).sum(1) for q in range(NCLS)], 1)  # [B_REAL, 4]
    order = np.argsort(-cnt.max(1), kind="stable")
    bin_cnt = np.zeros((TILES, NCLS), np.int32)
    bin_rows = np.zeros(TILES, np.int32)
    assign = np.empty(B_REAL, np.int32)
    for r in order:
        c4 = cnt[r]
        cand = bin_cnt + c4
        feas = (bin_rows < P) & (cand <= NI).all(1)
        if not feas.any():
            raise ValueError("tile balancing infeasible")
        load = cand.max(1).astype(np.float64) + bin_rows / 1000.0
        load[~feas] = 1e9
        b = int(load.argmin())
        assign[r] = b
        bin_cnt[b] += c4
        bin_rows[b] += 1
    # device position: rows of bin b occupy b*128.. in assignment order
    pos = np.empty(B_REAL, np.int64)
    next_slot = (np.arange(TILES) * P).astype(np.int64)
    for r in range(B_REAL):
        b = assign[r]
        pos[r] = next_slot[b]
        next_slot[b] += 1
    return pos


def _prep_core(shard):
    """shard: [B_REAL, K] int64 indices -> (idx16, sel, counts, pos)."""
    fp8_np = mybir.dt.np(FP8)
    pos = _balance(shard)
    idx16 = np.zeros((P, N_OPS * COLS), np.int16)
    sel = np.zeros((TILES, P, SLOTS * P), np.float32)
    cnts = np.zeros(N_OPS, np.int32)

    dev_tile = pos // P          # tile of each real row
    dev_loc = pos % P            # row-within-tile
    cls = (shard & 3).astype(np.int8)          # [B_REAL, K]
    loc16 = (shard >> 2).astype(np.int16)
    for t in range(TILES):
        rmask = dev_tile == t
        rids = np.nonzero(rmask)[0]
        for q in range(NCLS):
            rr, kk = np.nonzero(cls[rids] == q)
            n = rr.size
            if n > NI:
                raise ValueError(f"class overflow: tile {t} class {q}: {n}")
            loc = loc16[rids[rr], kk]
            rows_local = dev_loc[rids[rr]]
            order = np.argsort(loc, kind="stable")
            loc, rows_local = loc[order], rows_local[order]
            g = np.full(NI, -1, np.int16)
            g[:n] = loc
            op = t * NCLS + q
            cnts[op] = n
            idx16[:, op * COLS : (op + 1) * COLS] = np.tile(
                g.reshape(COLS, 16).T, (8, 1)
            )
            i = np.arange(n)
            sg = q * SLOTS_PER_CLS + i // P
            sel[t, i % P, sg * P + rows_local] = 1.0
    return idx16, sel.astype(fp8_np), cnts.reshape(1, N_OPS), pos


def _prep_all(neigh_idx):
    idx = np.asarray(neigh_idx).astype(np.int64)
    pad = np.broadcast_to(
        np.arange(K, dtype=np.int64) % NCLS, (N_CORES * B_REAL, K)
    ).copy()
    pad[:B] = idx
    shards = pad.reshape(N_CORES, B_REAL, K)
    in_maps, poss = [], []
    for c in range(N_CORES):
        idx16, sel, cnts, pos = _prep_core(shards[c])
        in_maps.append({"idx16": idx16, "sel": sel, "counts": cnts})
        poss.append(pos)
    return in_maps, poss


def make_in_maps(neigh_idx, embed_matrix):
    embed = np.ascontiguousarray(
        np.asarray(embed_matrix).astype(ml_dtypes.bfloat16)
    )
    in_maps, _ = _prep_all(neigh_idx)
    for m in in_maps:
        m["embed"] = embed
    return in_maps


def kernel(neigh_idx, embed_matrix):
    nc = _get_nc()
    embed = np.ascontiguousarray(
        np.asarray(embed_matrix).astype(ml_dtypes.bfloat16)
    )
    in_maps, poss = _prep_all(neigh_idx)
    for m in in_maps:
        m["embed"] = embed
    res = _run(nc, in_maps).results
    out = np.empty((B, D), np.float32)
    for c in range(N_CORES):
        dev = np.asarray(res[c]["out"]).astype(np.float32)   # [B_SHARD, D]
        lo, hi = c * B_REAL, min((c + 1) * B_REAL, B)
        out[lo:hi] = dev[poss[c][: hi - lo]]
    return out
